# revision 26
# baseline (speedup 1.0000x reference)
"""Trainium2 Bass kernel for nn_APIHyperInputLayer (hypernet MLP, 8-core data parallel).

Math (per branch):
    h   = prelu(F @ W1 + b1, alpha)                       [R, 64]
    w   = (h @ W2 + b2).reshape(R, F, 128)
    hid = einsum('rf,rfo->ro', F, w)
    out = hid.reshape(E, n, 128).sum(1)                   [E, 128]

Restructured: S[k,e,f] = sum_i h[(e,i),k] F[(e,i),f]; out[e,o] =
sum_{k,f} S[k,e,f] W2[k,f*128+o] + (bias term, computed on host).

v10 schedule (row-major h; no transposes; el-major fused M):
  Episodes padded to PITCH=16 rows; 8 episodes = one 128-partition group;
  within-group row->partition permutation p = 8*i + el spreads each el's
  16 rows over stride-8 partitions (better SDMA engine coverage).
  A: per group g, matmul([128 rows, 128 k], lhsT=fsp[:, g*128:+128],
     rhs=w1ext[0:81]); fsp has a ones-row so the bias rides the matmul.
     PReLU (pure, alpha) evacuates PSUM->SBUF per 4 groups.
  M: fused [128, 8*2560] tile; memsets zero it in 4 el-aligned pieces;
     8 clean el-DMAs from the compact host array fx write the diagonal.
  B: per group, two 64-part matmuls (f-pair lo/hi in partition halves)
     per branch; 4 ally groups or 8 enemy groups share one 2-bank PSUM
     tile; evac casts PSUM->s2[kk, fp*256+e] (vector 2/3, scalar 1/3).
  C: 40 accumulating matmuls out_T[o,e] += W2pair_fp.T @ s2 slice; the
     slices living in the early-loaded w2 half run first.
Output per core: [128 o, 256 e] fp32; host transposes/concats and adds the
bias term fsum @ b2 (host numpy, exact fp32).
"""

import os
import sys
import functools

import numpy as np

for _p in ("/opt/trn_rl_repo", os.path.expanduser("~/.axon_site/_ro/trn_rl_repo")):
    if os.path.isdir(_p) and _p not in sys.path:
        sys.path.insert(0, _p)

import dataclasses

import ml_dtypes

import concourse.bass as bass
import concourse.bacc as bacc
import concourse.mybir as mybir
import concourse.tile as tile
from concourse.bass_utils import run_bass_kernel_spmd

BF16 = mybir.dt.bfloat16
F32 = mybir.dt.float32

# Problem constants (hardcoded per contest rules)
N_CORES = 8
N_AGENTS, N_ENEMIES = 10, 11
ALLY_F, ENEMY_F = 48, 32
HYPER = 64
OUT = 128
B_FULL = 2048
E_C = B_FULL // N_CORES            # episodes per core = 256

PITCH = 16                         # padded rows per episode
EPG = 8                            # episodes per group (8*16=128 partitions)
NG = E_C // EPG                    # 32 groups
PROWS = E_C * PITCH                # padded rows per core = 4096
PAIR_A = ALLY_F // 2               # 24
PAIR_E = ENEMY_F // 2              # 16
S2A_FREE = PAIR_A * E_C            # 6144
S2E_FREE = PAIR_E * E_C            # 4096
W2COLS = (PAIR_A + PAIR_E) * OUT   # 5120
KROWS = ALLY_F + ENEMY_F + 1       # 81 = stacked features + ones row
ELW = NG * ALLY_F + NG * ENEMY_F   # 2560 = fused el-block width (ally|enemy)
M_FREE = EPG * ELW                 # 20480 = fused M tile free dim
EOFF = NG * ALLY_F                 # 1536 = enemy col offset within el block
SUBG = 4                           # groups per psA tile / prelu


def _perm_p(el, i):
    """Within-group row -> partition: p = 8*i + el (stride-8 spread)."""
    return 8 * i + el


PERM_ROW = [None] * 128
for _el in range(EPG):
    for _i in range(PITCH):
        PERM_ROW[_perm_p(_el, _i)] = (_el, _i)


def _ap(t, offset, dims):
    """Custom flat AP: dims = [(step, num), ...]; t is an AP or tensor handle."""
    a = t if isinstance(t, bass.AP) else t.ap()
    return dataclasses.replace(a, offset=offset, ap=[[s, n] for (s, n) in dims])


def build_program(alpha_a=0.25, alpha_e=0.25):
    assert alpha_a == alpha_e, "branches must share alpha"
    nc = bacc.Bacc("TRN2", target_bir_lowering=False, debug=False)

    fsp = nc.declare_dram_parameter("fsp", [KROWS, PROWS], BF16, isOutput=False)
    fx = nc.declare_dram_parameter("fx", [128, ELW], BF16, isOutput=False)
    w1e = nc.declare_dram_parameter("w1e", [128, 128], BF16, isOutput=False)
    w2pack = nc.declare_dram_parameter("w2pack", [128, W2COLS], BF16, isOutput=False)
    out_d = nc.declare_dram_parameter("out", [OUT, E_C], F32, isOutput=True)

    with tile.TileContext(nc) as tc:
        _emit(nc, tc, fsp, fx, w1e, w2pack, out_d, alpha_a)
    nc.compile()
    return nc


def _emit(nc, tc, fsp, fx, w1e, w2pack, out_d, alpha):
    from contextlib import ExitStack

    Prelu = mybir.ActivationFunctionType.Prelu

    ctx = ExitStack()
    with ctx:
        const = ctx.enter_context(tc.tile_pool(name="const", bufs=1))
        work = ctx.enter_context(tc.tile_pool(name="work", bufs=1))
        psA = ctx.enter_context(tc.tile_pool(name="psA", bufs=3, space="PSUM"))
        psB = ctx.enter_context(tc.tile_pool(name="psB", bufs=2, space="PSUM"))
        psC = ctx.enter_context(tc.tile_pool(name="psC", bufs=1, space="PSUM"))

        # ---- persistent SBUF ----
        fsp_sb = const.tile([KROWS, PROWS], BF16)
        w1_sb = const.tile([128, 128], BF16)
        w2_sb = const.tile([128, W2COLS], BF16)
        h_sb = work.tile([128, PROWS], BF16)
        m_sb = work.tile([128, M_FREE], BF16)
        s2a = work.tile([128, S2A_FREE], BF16)
        s2e = work.tile([128, S2E_FREE], BF16)
        osb = work.tile([OUT, E_C], F32)

        # ---- PE warmup: junk matmuls to flip the HAM clock gate to 8/8
        # before real work arrives (reads a memset junk tile) ----
        junk = work.tile([128, 64], BF16)
        nc.vector.memset(junk[:].bitcast(F32), 0.0)
        pw = psC.tile([OUT, E_C], F32, tag="psC")
        for wi in range(80):
            nc.tensor.matmul(pw[0:64, 0:64], junk[:], junk[:],
                             start=True, stop=True)

        # ---- SWDGE row takes both w2 halves first thing (only stage C
        # needs them; keeps the big transfers off the sync/scalar rings) ----
        nc.gpsimd.dma_start(w2_sb[:, W2COLS // 2 :], w2pack.ap()[:, W2COLS // 2 :])
        nc.gpsimd.dma_start(w2_sb[:, 0 : W2COLS // 2], w2pack.ap()[:, 0 : W2COLS // 2])

        # ---- memsets for M off-diagonal zeros, 4 el-aligned pieces so
        # the el-DMAs can chase them (vector + gpsimd, f32 view) ----
        m_f32 = m_sb[:].bitcast(F32)
        P4 = M_FREE // 8            # f32 cols per piece (2 el blocks)
        nc.vector.memset(m_f32[:, 0 * P4 : 1 * P4], 0.0)   # els 0,1
        nc.gpsimd.memset(m_f32[:, 1 * P4 : 2 * P4], 0.0)   # els 2,3
        nc.vector.memset(m_f32[:, 2 * P4 : 3 * P4], 0.0)   # els 4,5
        nc.gpsimd.memset(m_f32[:, 3 * P4 : 4 * P4], 0.0)   # els 6,7
        # fsp chunks: 2 on the sync row, 2 on the scalar row (after w1).
        CHW = PROWS // 4
        nc.sync.dma_start(fsp_sb[:, 0:CHW], fsp.ap()[:, 0:CHW])
        nc.sync.dma_start(fsp_sb[:, CHW : 2 * CHW], fsp.ap()[:, CHW : 2 * CHW])
        nc.scalar.dma_start(w1_sb[:], w1e.ap())
        nc.scalar.dma_start(
            fsp_sb[:, 2 * CHW : 3 * CHW], fsp.ap()[:, 2 * CHW : 3 * CHW])
        nc.scalar.dma_start(
            fsp_sb[:, 3 * CHW : 4 * CHW], fsp.ap()[:, 3 * CHW : 4 * CHW])

        # fused M diag el-DMAs: one per el; dst partitions are the
        # permuted el rows (stride-8 partition dim), cols contiguous.
        def el_dma(eng, el):
            eng.dma_start(
                _ap(m_sb, el * (M_FREE + ELW), [
                    (8 * M_FREE, PITCH),    # i -> partition 8i+el
                    (1, ELW),
                ]),
                fx.ap()[el * PITCH : (el + 1) * PITCH, :])

        for el in range(EPG):
            el_dma(nc.sync, el)

        # ---- pipeline ----
        copy_rr = [0]

        def evac(dst, src):
            if copy_rr[0] % 3 == 2:
                nc.scalar.copy(dst, src)
            else:
                nc.vector.tensor_copy(dst, src)
            copy_rr[0] += 1

        def a_stage(sub):
            pa = psA.tile([128, SUBG * 128], F32, tag="psA")
            for j in range(SUBG):
                g = sub * SUBG + j
                nc.tensor.matmul(
                    pa[:, j * 128 : (j + 1) * 128],
                    fsp_sb[:, g * 128 : (g + 1) * 128],
                    w1_sb[0:KROWS, :],
                    start=True, stop=True)
            nc.scalar.activation(
                h_sb[:, sub * 512 : (sub + 1) * 512], pa[:],
                Prelu, scale=1.0, alpha=alpha)

        def b_ally(g0):              # 4 groups per 2-bank PSUM tile
            pb = psB.tile([128, 1024], F32, tag="psB")
            for u in range(4):
                g = g0 + u
                lhsT = h_sb[:, g * 128 : g * 128 + 64]
                nc.tensor.matmul(
                    pb[64:128, u * 192 : (u + 1) * 192], lhsT,
                    _ap(m_sb, g * ALLY_F + PAIR_A,
                        [(M_FREE, 128), (ELW, EPG), (1, PAIR_A)]),
                    start=True, stop=True)
                nc.tensor.matmul(
                    pb[0:64, u * 192 : (u + 1) * 192], lhsT,
                    _ap(m_sb, g * ALLY_F,
                        [(M_FREE, 128), (ELW, EPG), (1, PAIR_A)]),
                    start=True, stop=True)
            evac(
                _ap(s2a, g0 * EPG,
                    [(S2A_FREE, 128), (E_C, PAIR_A), (1, 4 * EPG)]),
                _ap(pb, 0,
                    [(1024, 128), (1, PAIR_A), (192, 4), (PAIR_A, EPG)]))

        def b_enemy(g0):             # 8 groups per 2-bank PSUM tile
            pb = psB.tile([128, 1024], F32, tag="psB")
            for u in range(8):
                g = g0 + u
                lhsT = h_sb[:, g * 128 + 64 : g * 128 + 128]
                nc.tensor.matmul(
                    pb[64:128, u * 128 : (u + 1) * 128], lhsT,
                    _ap(m_sb, EOFF + g * ENEMY_F + PAIR_E,
                        [(M_FREE, 128), (ELW, EPG), (1, PAIR_E)]),
                    start=True, stop=True)
                nc.tensor.matmul(
                    pb[0:64, u * 128 : (u + 1) * 128], lhsT,
                    _ap(m_sb, EOFF + g * ENEMY_F,
                        [(M_FREE, 128), (ELW, EPG), (1, PAIR_E)]),
                    start=True, stop=True)
            evac(
                _ap(s2e, g0 * EPG,
                    [(S2E_FREE, 128), (E_C, PAIR_E), (1, 8 * EPG)]),
                _ap(pb, 0,
                    [(1024, 128), (1, PAIR_E), (128, 8), (PAIR_E, EPG)]))

        for sub in range(NG // SUBG):
            a_stage(sub)
        # filler junk matmuls into the warmup tile: keep the PE busy (HAM
        # warm) while the last el-DMAs land before stage B
        for wi in range(24):
            nc.tensor.matmul(pw[0:64, 0:64], junk[:], junk[:],
                             start=True, stop=True)
        for g0 in range(0, NG, 4):
            b_ally(g0)
        for g0 in range(0, NG, 8):
            b_enemy(g0)

        # ---- stage C: out_T[o,e] accumulation over 40 fp slices; slices
        # in the early-loaded w2 half (cols 2560:) run first ----
        pc = psC.tile([OUT, E_C], F32, tag="psC")
        n_sl = PAIR_A + PAIR_E
        order = ([("a", fp) for fp in range(PAIR_A)]
                 + [("e", fp) for fp in range(PAIR_E)])
        for idx, (br, fp) in enumerate(order):
            if br == "a":
                lhsT = w2_sb[:, fp * OUT : (fp + 1) * OUT]
                rhs = s2a[:, fp * E_C : (fp + 1) * E_C]
            else:
                lhsT = w2_sb[:, (PAIR_A + fp) * OUT : (PAIR_A + fp + 1) * OUT]
                rhs = s2e[:, fp * E_C : (fp + 1) * E_C]
            nc.tensor.matmul(pc[:], lhsT, rhs,
                             start=(idx == 0), stop=(idx == n_sl - 1))

        nc.vector.tensor_copy(osb[:], pc[:])
        nc.sync.dma_start(out_d.ap(), osb[:])


@functools.lru_cache(maxsize=2)
def _cached_program(alpha_a, alpha_e):
    return build_program(alpha_a, alpha_e)


def host_prep(ally_features, enemy_features, Wa1, ba1, aa, Wa2, ba2,
              We1, be1, ae, We2, be2):
    """Per-core input maps (numpy, bf16) + host-side bias term."""
    bf = ml_dtypes.bfloat16

    def uniform_alpha(a):
        a = np.asarray(a, dtype=np.float32)
        assert np.allclose(a, a[0]), "per-channel alpha not supported"
        return float(a[0])

    ua, ue = uniform_alpha(aa), uniform_alpha(ae)

    w1 = np.zeros((128, 128), dtype=np.float32)
    w1[0:ALLY_F, 0:HYPER] = np.asarray(Wa1)
    w1[ALLY_F:80, HYPER:128] = np.asarray(We1)
    w1[80, 0:HYPER] = np.asarray(ba1)
    w1[80, HYPER:128] = np.asarray(be1)
    w1 = w1.astype(bf)

    w2 = np.zeros((128, W2COLS), dtype=np.float32)
    Wa2_, We2_ = np.asarray(Wa2), np.asarray(We2)
    for fp in range(PAIR_A):
        w2[0:HYPER, fp * OUT : (fp + 1) * OUT] = Wa2_[:, fp * OUT : (fp + 1) * OUT]
        w2[HYPER:128, fp * OUT : (fp + 1) * OUT] = \
            Wa2_[:, (fp + PAIR_A) * OUT : (fp + PAIR_A + 1) * OUT]
    for fp in range(PAIR_E):
        c = (PAIR_A + fp) * OUT
        w2[0:HYPER, c : c + OUT] = We2_[:, fp * OUT : (fp + 1) * OUT]
        w2[HYPER:128, c : c + OUT] = \
            We2_[:, (fp + PAIR_E) * OUT : (fp + PAIR_E + 1) * OUT]
    w2 = w2.astype(bf)

    fa_all = np.asarray(ally_features, dtype=np.float32)
    fe_all = np.asarray(enemy_features, dtype=np.float32)
    fa_bf = fa_all.astype(bf)
    fe_bf = fe_all.astype(bf)

    # host-side bias term: fsum @ b2 (exact fp32)
    fsum_a = fa_all.reshape(B_FULL, N_AGENTS, ALLY_F).sum(axis=1)
    fsum_e = fe_all.reshape(B_FULL, N_ENEMIES, ENEMY_F).sum(axis=1)
    bias_out = (fsum_a @ np.asarray(ba2).reshape(ALLY_F, OUT)
                + fsum_e @ np.asarray(be2).reshape(ENEMY_F, OUT)).astype(np.float32)

    RA = E_C * N_AGENTS
    RE = E_C * N_ENEMIES
    in_maps = []
    for c in range(N_CORES):
        fa_c = fa_bf[c * RA : (c + 1) * RA].reshape(E_C, N_AGENTS, ALLY_F)
        fe_c = fe_bf[c * RE : (c + 1) * RE].reshape(E_C, N_ENEMIES, ENEMY_F)
        fa_pad = np.zeros((E_C, PITCH, ALLY_F), dtype=bf)
        fa_pad[:, :N_AGENTS, :] = fa_c
        fe_pad = np.zeros((E_C, PITCH, ENEMY_F), dtype=bf)
        fe_pad[:, :N_ENEMIES, :] = fe_c

        # group-row layouts under the partition permutation:
        # within group g, partition p holds episode-row (el,i) = PERM_ROW[p]
        def group_rows(f_pad, featf):
            f5 = f_pad.reshape(NG, EPG, PITCH, featf)
            out = np.empty((NG, 128, featf), dtype=bf)
            for p in range(128):
                el, i = PERM_ROW[p]
                out[:, p, :] = f5[:, el, i, :]
            return out

        ga = group_rows(fa_pad, ALLY_F)          # [NG, 128, 48]
        ge = group_rows(fe_pad, ENEMY_F)         # [NG, 128, 32]

        # fsp: [81, PROWS], col g*128 + p
        fs = np.zeros((KROWS, PROWS), dtype=bf)
        fs[0:ALLY_F, :] = ga.transpose(2, 0, 1).reshape(ALLY_F, PROWS)
        fs[ALLY_F:80, :] = ge.transpose(2, 0, 1).reshape(ENEMY_F, PROWS)
        fs[80, :] = np.float32(1.0)

        # fx: compact el-major diag source; row el*16 + i
        def m_compact(f_pad, featf):
            f5 = f_pad.reshape(NG, EPG, PITCH, featf)
            out = np.empty((128, NG * featf), dtype=bf)
            for el in range(EPG):
                for i in range(PITCH):
                    out[el * PITCH + i] = f5[:, el, i, :].reshape(NG * featf)
            return out

        in_maps.append({
            "fsp": np.ascontiguousarray(fs),
            "fx": np.ascontiguousarray(np.concatenate(
                [m_compact(fa_pad, ALLY_F), m_compact(fe_pad, ENEMY_F)],
                axis=1)),
            "w1e": w1, "w2pack": w2,
        })
    aux = {"bias_out": bias_out, "ua": ua, "ue": ue}
    return in_maps, aux


def assemble_output(results, aux):
    outs = [np.asarray(r["out"], dtype=np.float32) for r in results]
    dev = np.concatenate([o.T for o in outs], axis=0)
    return dev + aux["bias_out"]


def kernel(**inputs) -> np.ndarray:
    in_maps, aux = host_prep(**inputs)
    nc = _cached_program(aux["ua"], aux["ue"])
    res = run_bass_kernel_spmd(nc, in_maps, core_ids=list(range(N_CORES)))
    return assemble_output(res.results, aux)


if __name__ == "__main__":
    build_program()
    print("built ok")


# revision 27
# speedup vs baseline: 1.0441x; 1.0441x over previous
"""Trainium2 Bass kernel for nn_APIHyperInputLayer (hypernet MLP, 8-core data parallel).

Math (per branch):
    h   = prelu(F @ W1 + b1, alpha)                       [R, 64]
    w   = (h @ W2 + b2).reshape(R, F, 128)
    hid = einsum('rf,rfo->ro', F, w)
    out = hid.reshape(E, n, 128).sum(1)                   [E, 128]

Restructured: S[k,e,f] = sum_i h[(e,i),k] F[(e,i),f]; out[e,o] =
sum_{k,f} S[k,e,f] W2[k,f*128+o] + (bias term, computed on host).

v10 schedule (row-major h; no transposes; el-major fused M):
  Episodes padded to PITCH=16 rows; 8 episodes = one 128-partition group;
  within-group row->partition permutation p = 8*i + el spreads each el's
  16 rows over stride-8 partitions (better SDMA engine coverage).
  A: per group g, matmul([128 rows, 128 k], lhsT=fsp[:, g*128:+128],
     rhs=w1ext[0:81]); fsp has a ones-row so the bias rides the matmul.
     PReLU (pure, alpha) evacuates PSUM->SBUF per 4 groups.
  M: fused [128, 8*2560] tile; memsets zero it in 4 el-aligned pieces;
     8 clean el-DMAs from the compact host array fx write the diagonal.
  B: per group, two 64-part matmuls (f-pair lo/hi in partition halves)
     per branch; 4 ally groups or 8 enemy groups share one 2-bank PSUM
     tile; evac casts PSUM->s2[kk, fp*256+e] (vector 2/3, scalar 1/3).
  C: 40 accumulating matmuls out_T[o,e] += W2pair_fp.T @ s2 slice; the
     slices living in the early-loaded w2 half run first.
Output per core: [128 o, 256 e] fp32; host transposes/concats and adds the
bias term fsum @ b2 (host numpy, exact fp32).
"""

import os
import sys
import functools

import numpy as np

for _p in ("/opt/trn_rl_repo", os.path.expanduser("~/.axon_site/_ro/trn_rl_repo")):
    if os.path.isdir(_p) and _p not in sys.path:
        sys.path.insert(0, _p)

import dataclasses

import ml_dtypes

import concourse.bass as bass
import concourse.bacc as bacc
import concourse.mybir as mybir
import concourse.tile as tile
from concourse.bass_utils import run_bass_kernel_spmd

BF16 = mybir.dt.bfloat16
F32 = mybir.dt.float32

# Problem constants (hardcoded per contest rules)
N_CORES = 8
N_AGENTS, N_ENEMIES = 10, 11
ALLY_F, ENEMY_F = 48, 32
HYPER = 64
OUT = 128
B_FULL = 2048
E_C = B_FULL // N_CORES            # episodes per core = 256

PITCH = 16                         # padded rows per episode
EPG = 8                            # episodes per group (8*16=128 partitions)
NG = E_C // EPG                    # 32 groups
PROWS = E_C * PITCH                # padded rows per core = 4096
PAIR_A = ALLY_F // 2               # 24
PAIR_E = ENEMY_F // 2              # 16
S2A_FREE = PAIR_A * E_C            # 6144
S2E_FREE = PAIR_E * E_C            # 4096
W2COLS = (PAIR_A + PAIR_E) * OUT   # 5120
KROWS = ALLY_F + ENEMY_F + 1       # 81 = stacked features + ones row
ELW = NG * ALLY_F + NG * ENEMY_F   # 2560 = fused el-block width (ally|enemy)
M_FREE = EPG * ELW                 # 20480 = fused M tile free dim
EOFF = NG * ALLY_F                 # 1536 = enemy col offset within el block
SUBG = 4                           # groups per psA tile / prelu


def _perm_p(el, i):
    """Within-group row -> partition: p = 8*i + el (stride-8 spread)."""
    return 8 * i + el


PERM_ROW = [None] * 128
for _el in range(EPG):
    for _i in range(PITCH):
        PERM_ROW[_perm_p(_el, _i)] = (_el, _i)


def _ap(t, offset, dims):
    """Custom flat AP: dims = [(step, num), ...]; t is an AP or tensor handle."""
    a = t if isinstance(t, bass.AP) else t.ap()
    return dataclasses.replace(a, offset=offset, ap=[[s, n] for (s, n) in dims])


def build_program(alpha_a=0.25, alpha_e=0.25):
    assert alpha_a == alpha_e, "branches must share alpha"
    nc = bacc.Bacc("TRN2", target_bir_lowering=False, debug=False)

    fsp = nc.declare_dram_parameter("fsp", [KROWS, PROWS], BF16, isOutput=False)
    fx = nc.declare_dram_parameter("fx", [128, ELW], BF16, isOutput=False)
    w1e = nc.declare_dram_parameter("w1e", [128, 128], BF16, isOutput=False)
    w2pack = nc.declare_dram_parameter("w2pack", [128, W2COLS], BF16, isOutput=False)
    out_d = nc.declare_dram_parameter("out", [OUT, E_C], F32, isOutput=True)

    with tile.TileContext(nc) as tc:
        _emit(nc, tc, fsp, fx, w1e, w2pack, out_d, alpha_a)
    nc.compile()
    return nc


def _emit(nc, tc, fsp, fx, w1e, w2pack, out_d, alpha):
    from contextlib import ExitStack

    Prelu = mybir.ActivationFunctionType.Prelu

    ctx = ExitStack()
    with ctx:
        const = ctx.enter_context(tc.tile_pool(name="const", bufs=1))
        work = ctx.enter_context(tc.tile_pool(name="work", bufs=1))
        psA = ctx.enter_context(tc.tile_pool(name="psA", bufs=3, space="PSUM"))
        psB = ctx.enter_context(tc.tile_pool(name="psB", bufs=2, space="PSUM"))
        psC = ctx.enter_context(tc.tile_pool(name="psC", bufs=1, space="PSUM"))

        # ---- persistent SBUF ----
        fsp_sb = const.tile([KROWS, PROWS], BF16)
        w1_sb = const.tile([128, 128], BF16)
        w2_sb = const.tile([128, W2COLS], BF16)
        h_sb = work.tile([128, PROWS], BF16)
        m_sb = work.tile([128, M_FREE], BF16)
        s2a = work.tile([128, S2A_FREE], BF16)
        s2e = work.tile([128, S2E_FREE], BF16)
        osb = work.tile([OUT, E_C], F32)

        # ---- PE warmup: junk matmuls to flip the HAM clock gate to 8/8
        # before real work arrives (reads a memset junk tile) ----
        junk = work.tile([128, 64], BF16)
        nc.vector.memset(junk[:].bitcast(F32), 0.0)
        pw = psC.tile([OUT, E_C], F32, tag="psC")
        for wi in range(80):
            nc.tensor.matmul(pw[0:64, 0:64], junk[:], junk[:],
                             start=True, stop=True)

        # ---- SWDGE row takes both w2 halves first thing (only stage C
        # needs them; keeps the big transfers off the sync/scalar rings) ----
        nc.gpsimd.dma_start(w2_sb[:, W2COLS // 2 :], w2pack.ap()[:, W2COLS // 2 :])
        nc.gpsimd.dma_start(w2_sb[:, 0 : W2COLS // 2], w2pack.ap()[:, 0 : W2COLS // 2])

        # ---- memsets for M off-diagonal zeros, 4 el-aligned pieces so
        # the el-DMAs can chase them (vector + gpsimd, f32 view) ----
        m_f32 = m_sb[:].bitcast(F32)
        P4 = M_FREE // 8            # f32 cols per piece (2 el blocks)
        nc.vector.memset(m_f32[:, 0 * P4 : 1 * P4], 0.0)   # els 0,1
        nc.gpsimd.memset(m_f32[:, 1 * P4 : 2 * P4], 0.0)   # els 2,3
        nc.vector.memset(m_f32[:, 2 * P4 : 3 * P4], 0.0)   # els 4,5
        nc.gpsimd.memset(m_f32[:, 3 * P4 : 4 * P4], 0.0)   # els 6,7
        # fsp chunks: 2 on the sync row, 2 on the scalar row (after w1).
        CHW = PROWS // 4
        nc.sync.dma_start(fsp_sb[:, 0:CHW], fsp.ap()[:, 0:CHW])
        nc.sync.dma_start(fsp_sb[:, CHW : 2 * CHW], fsp.ap()[:, CHW : 2 * CHW])
        nc.scalar.dma_start(w1_sb[:], w1e.ap())
        nc.scalar.dma_start(
            fsp_sb[:, 2 * CHW : 3 * CHW], fsp.ap()[:, 2 * CHW : 3 * CHW])
        nc.scalar.dma_start(
            fsp_sb[:, 3 * CHW : 4 * CHW], fsp.ap()[:, 3 * CHW : 4 * CHW])

        # fused M diag el-DMAs: one per el; dst partitions are the
        # permuted el rows (stride-8 partition dim), cols contiguous.
        def el_dma(eng, el):
            eng.dma_start(
                _ap(m_sb, el * (M_FREE + ELW), [
                    (8 * M_FREE, PITCH),    # i -> partition 8i+el
                    (1, ELW),
                ]),
                fx.ap()[el * PITCH : (el + 1) * PITCH, :])

        for el in range(EPG):
            el_dma(nc.sync, el)

        # ---- pipeline ----
        copy_rr = [0]

        def evac(dst, src):
            if copy_rr[0] % 3 == 2:
                nc.scalar.copy(dst, src)
            else:
                nc.vector.tensor_copy(dst, src)
            copy_rr[0] += 1

        def a_stage(sub):
            pa = psA.tile([128, SUBG * 128], F32, tag="psA")
            for j in range(SUBG):
                g = sub * SUBG + j
                nc.tensor.matmul(
                    pa[:, j * 128 : (j + 1) * 128],
                    fsp_sb[:, g * 128 : (g + 1) * 128],
                    w1_sb[0:KROWS, :],
                    start=True, stop=True)
            nc.scalar.activation(
                h_sb[:, sub * 512 : (sub + 1) * 512], pa[:],
                Prelu, scale=1.0, alpha=alpha)

        def b_ally(g0):              # 4 groups per 2-bank PSUM tile
            pb = psB.tile([128, 1024], F32, tag="psB")
            for u in range(4):
                g = g0 + u
                lhsT = h_sb[:, g * 128 : g * 128 + 64]
                nc.tensor.matmul(
                    pb[64:128, u * 192 : (u + 1) * 192], lhsT,
                    _ap(m_sb, g * ALLY_F + PAIR_A,
                        [(M_FREE, 128), (ELW, EPG), (1, PAIR_A)]),
                    start=True, stop=True)
                nc.tensor.matmul(
                    pb[0:64, u * 192 : (u + 1) * 192], lhsT,
                    _ap(m_sb, g * ALLY_F,
                        [(M_FREE, 128), (ELW, EPG), (1, PAIR_A)]),
                    start=True, stop=True)
            evac(
                _ap(s2a, g0 * EPG,
                    [(S2A_FREE, 128), (E_C, PAIR_A), (1, 4 * EPG)]),
                _ap(pb, 0,
                    [(1024, 128), (1, PAIR_A), (192, 4), (PAIR_A, EPG)]))

        def b_enemy(g0):             # 8 groups per 2-bank PSUM tile
            pb = psB.tile([128, 1024], F32, tag="psB")
            for u in range(8):
                g = g0 + u
                lhsT = h_sb[:, g * 128 + 64 : g * 128 + 128]
                nc.tensor.matmul(
                    pb[64:128, u * 128 : (u + 1) * 128], lhsT,
                    _ap(m_sb, EOFF + g * ENEMY_F + PAIR_E,
                        [(M_FREE, 128), (ELW, EPG), (1, PAIR_E)]),
                    start=True, stop=True)
                nc.tensor.matmul(
                    pb[0:64, u * 128 : (u + 1) * 128], lhsT,
                    _ap(m_sb, EOFF + g * ENEMY_F,
                        [(M_FREE, 128), (ELW, EPG), (1, PAIR_E)]),
                    start=True, stop=True)
            evac(
                _ap(s2e, g0 * EPG,
                    [(S2E_FREE, 128), (E_C, PAIR_E), (1, 8 * EPG)]),
                _ap(pb, 0,
                    [(1024, 128), (1, PAIR_E), (128, 8), (PAIR_E, EPG)]))

        for sub in range(NG // SUBG):
            a_stage(sub)
        for g0 in range(0, NG, 4):
            b_ally(g0)
        for g0 in range(0, NG, 8):
            b_enemy(g0)

        # ---- stage C: out_T[o,e] accumulation over 40 fp slices; slices
        # in the early-loaded w2 half (cols 2560:) run first ----
        pc = psC.tile([OUT, E_C], F32, tag="psC")
        n_sl = PAIR_A + PAIR_E
        order = ([("a", fp) for fp in range(PAIR_A)]
                 + [("e", fp) for fp in range(PAIR_E)])
        for idx, (br, fp) in enumerate(order):
            if br == "a":
                lhsT = w2_sb[:, fp * OUT : (fp + 1) * OUT]
                rhs = s2a[:, fp * E_C : (fp + 1) * E_C]
            else:
                lhsT = w2_sb[:, (PAIR_A + fp) * OUT : (PAIR_A + fp + 1) * OUT]
                rhs = s2e[:, fp * E_C : (fp + 1) * E_C]
            nc.tensor.matmul(pc[:], lhsT, rhs,
                             start=(idx == 0), stop=(idx == n_sl - 1))

        nc.vector.tensor_copy(osb[:], pc[:])
        nc.sync.dma_start(out_d.ap(), osb[:])


@functools.lru_cache(maxsize=2)
def _cached_program(alpha_a, alpha_e):
    return build_program(alpha_a, alpha_e)


def host_prep(ally_features, enemy_features, Wa1, ba1, aa, Wa2, ba2,
              We1, be1, ae, We2, be2):
    """Per-core input maps (numpy, bf16) + host-side bias term."""
    bf = ml_dtypes.bfloat16

    def uniform_alpha(a):
        a = np.asarray(a, dtype=np.float32)
        assert np.allclose(a, a[0]), "per-channel alpha not supported"
        return float(a[0])

    ua, ue = uniform_alpha(aa), uniform_alpha(ae)

    w1 = np.zeros((128, 128), dtype=np.float32)
    w1[0:ALLY_F, 0:HYPER] = np.asarray(Wa1)
    w1[ALLY_F:80, HYPER:128] = np.asarray(We1)
    w1[80, 0:HYPER] = np.asarray(ba1)
    w1[80, HYPER:128] = np.asarray(be1)
    w1 = w1.astype(bf)

    w2 = np.zeros((128, W2COLS), dtype=np.float32)
    Wa2_, We2_ = np.asarray(Wa2), np.asarray(We2)
    for fp in range(PAIR_A):
        w2[0:HYPER, fp * OUT : (fp + 1) * OUT] = Wa2_[:, fp * OUT : (fp + 1) * OUT]
        w2[HYPER:128, fp * OUT : (fp + 1) * OUT] = \
            Wa2_[:, (fp + PAIR_A) * OUT : (fp + PAIR_A + 1) * OUT]
    for fp in range(PAIR_E):
        c = (PAIR_A + fp) * OUT
        w2[0:HYPER, c : c + OUT] = We2_[:, fp * OUT : (fp + 1) * OUT]
        w2[HYPER:128, c : c + OUT] = \
            We2_[:, (fp + PAIR_E) * OUT : (fp + PAIR_E + 1) * OUT]
    w2 = w2.astype(bf)

    fa_all = np.asarray(ally_features, dtype=np.float32)
    fe_all = np.asarray(enemy_features, dtype=np.float32)
    fa_bf = fa_all.astype(bf)
    fe_bf = fe_all.astype(bf)

    # host-side bias term: fsum @ b2 (exact fp32)
    fsum_a = fa_all.reshape(B_FULL, N_AGENTS, ALLY_F).sum(axis=1)
    fsum_e = fe_all.reshape(B_FULL, N_ENEMIES, ENEMY_F).sum(axis=1)
    bias_out = (fsum_a @ np.asarray(ba2).reshape(ALLY_F, OUT)
                + fsum_e @ np.asarray(be2).reshape(ENEMY_F, OUT)).astype(np.float32)

    RA = E_C * N_AGENTS
    RE = E_C * N_ENEMIES
    in_maps = []
    for c in range(N_CORES):
        fa_c = fa_bf[c * RA : (c + 1) * RA].reshape(E_C, N_AGENTS, ALLY_F)
        fe_c = fe_bf[c * RE : (c + 1) * RE].reshape(E_C, N_ENEMIES, ENEMY_F)
        fa_pad = np.zeros((E_C, PITCH, ALLY_F), dtype=bf)
        fa_pad[:, :N_AGENTS, :] = fa_c
        fe_pad = np.zeros((E_C, PITCH, ENEMY_F), dtype=bf)
        fe_pad[:, :N_ENEMIES, :] = fe_c

        # group-row layouts under the partition permutation:
        # within group g, partition p holds episode-row (el,i) = PERM_ROW[p]
        def group_rows(f_pad, featf):
            f5 = f_pad.reshape(NG, EPG, PITCH, featf)
            out = np.empty((NG, 128, featf), dtype=bf)
            for p in range(128):
                el, i = PERM_ROW[p]
                out[:, p, :] = f5[:, el, i, :]
            return out

        ga = group_rows(fa_pad, ALLY_F)          # [NG, 128, 48]
        ge = group_rows(fe_pad, ENEMY_F)         # [NG, 128, 32]

        # fsp: [81, PROWS], col g*128 + p
        fs = np.zeros((KROWS, PROWS), dtype=bf)
        fs[0:ALLY_F, :] = ga.transpose(2, 0, 1).reshape(ALLY_F, PROWS)
        fs[ALLY_F:80, :] = ge.transpose(2, 0, 1).reshape(ENEMY_F, PROWS)
        fs[80, :] = np.float32(1.0)

        # fx: compact el-major diag source; row el*16 + i
        def m_compact(f_pad, featf):
            f5 = f_pad.reshape(NG, EPG, PITCH, featf)
            out = np.empty((128, NG * featf), dtype=bf)
            for el in range(EPG):
                for i in range(PITCH):
                    out[el * PITCH + i] = f5[:, el, i, :].reshape(NG * featf)
            return out

        in_maps.append({
            "fsp": np.ascontiguousarray(fs),
            "fx": np.ascontiguousarray(np.concatenate(
                [m_compact(fa_pad, ALLY_F), m_compact(fe_pad, ENEMY_F)],
                axis=1)),
            "w1e": w1, "w2pack": w2,
        })
    aux = {"bias_out": bias_out, "ua": ua, "ue": ue}
    return in_maps, aux


def assemble_output(results, aux):
    outs = [np.asarray(r["out"], dtype=np.float32) for r in results]
    dev = np.concatenate([o.T for o in outs], axis=0)
    return dev + aux["bias_out"]


def kernel(**inputs) -> np.ndarray:
    in_maps, aux = host_prep(**inputs)
    nc = _cached_program(aux["ua"], aux["ue"])
    res = run_bass_kernel_spmd(nc, in_maps, core_ids=list(range(N_CORES)))
    return assemble_output(res.results, aux)


if __name__ == "__main__":
    build_program()
    print("built ok")


# revision 32
# speedup vs baseline: 1.0671x; 1.0221x over previous
"""Trainium2 Bass kernel for nn_APIHyperInputLayer (hypernet MLP, 8-core data parallel).

Math (per branch):
    h   = prelu(F @ W1 + b1, alpha)                       [R, 64]
    w   = (h @ W2 + b2).reshape(R, F, 128)
    hid = einsum('rf,rfo->ro', F, w)
    out = hid.reshape(E, n, 128).sum(1)                   [E, 128]

Restructured: S[k,e,f] = sum_i h[(e,i),k] F[(e,i),f]; out[e,o] =
sum_{k,f} S[k,e,f] W2[k,f*128+o] + (bias term, computed on host).

v10 schedule (row-major h; no transposes; el-major fused M):
  Episodes padded to PITCH=16 rows; 8 episodes = one 128-partition group;
  within-group row->partition permutation p = 8*i + el spreads each el's
  16 rows over stride-8 partitions (better SDMA engine coverage).
  A: per group g, matmul([128 rows, 128 k], lhsT=fsp[:, g*128:+128],
     rhs=w1ext[0:81]); fsp has a ones-row so the bias rides the matmul.
     PReLU (pure, alpha) evacuates PSUM->SBUF per 4 groups.
  M: fused [128, 8*2560] tile; memsets zero it in 4 el-aligned pieces;
     8 clean el-DMAs from the compact host array fx write the diagonal.
  B: per group, two 64-part matmuls (f-pair lo/hi in partition halves)
     per branch; 4 ally groups or 8 enemy groups share one 2-bank PSUM
     tile; evac casts PSUM->s2[kk, fp*256+e] (vector 2/3, scalar 1/3).
  C: 40 accumulating matmuls out_T[o,e] += W2pair_fp.T @ s2 slice; the
     slices living in the early-loaded w2 half run first.
Output per core: [128 o, 256 e] fp32; host transposes/concats and adds the
bias term fsum @ b2 (host numpy, exact fp32).
"""

import os
import sys
import functools

import numpy as np

for _p in ("/opt/trn_rl_repo", os.path.expanduser("~/.axon_site/_ro/trn_rl_repo")):
    if os.path.isdir(_p) and _p not in sys.path:
        sys.path.insert(0, _p)

import dataclasses

import ml_dtypes

import concourse.bass as bass
import concourse.bacc as bacc
import concourse.mybir as mybir
import concourse.tile as tile
from concourse.bass_utils import run_bass_kernel_spmd

BF16 = mybir.dt.bfloat16
F32 = mybir.dt.float32

# Problem constants (hardcoded per contest rules)
N_CORES = 8
N_AGENTS, N_ENEMIES = 10, 11
ALLY_F, ENEMY_F = 48, 32
HYPER = 64
OUT = 128
B_FULL = 2048
E_C = B_FULL // N_CORES            # episodes per core = 256

PITCH = 16                         # padded rows per episode
EPG = 8                            # episodes per group (8*16=128 partitions)
NG = E_C // EPG                    # 32 groups
PROWS = E_C * PITCH                # padded rows per core = 4096
PAIR_A = ALLY_F // 2               # 24
PAIR_E = ENEMY_F // 2              # 16
S2A_FREE = PAIR_A * E_C            # 6144
S2E_FREE = PAIR_E * E_C            # 4096
W2COLS = (PAIR_A + PAIR_E) * OUT   # 5120
KROWS = ALLY_F + ENEMY_F + 1       # 81 = stacked features + ones row
ELW = NG * ALLY_F + NG * ENEMY_F   # 2560 = fused el-block width (ally|enemy)
M_FREE = EPG * ELW                 # 20480 = fused M tile free dim
EOFF = NG * ALLY_F                 # 1536 = enemy col offset within el block
SUBG = 4                           # groups per psA tile / prelu


def _perm_p(el, i):
    """Within-group row -> partition: p = 8*i + el (stride-8 spread)."""
    return 8 * i + el


PERM_ROW = [None] * 128
for _el in range(EPG):
    for _i in range(PITCH):
        PERM_ROW[_perm_p(_el, _i)] = (_el, _i)


def _ap(t, offset, dims):
    """Custom flat AP: dims = [(step, num), ...]; t is an AP or tensor handle."""
    a = t if isinstance(t, bass.AP) else t.ap()
    return dataclasses.replace(a, offset=offset, ap=[[s, n] for (s, n) in dims])


def build_program(alpha_a=0.25, alpha_e=0.25):
    assert alpha_a == alpha_e, "branches must share alpha"
    nc = bacc.Bacc("TRN2", target_bir_lowering=False, debug=False)

    fsp = nc.declare_dram_parameter("fsp", [KROWS, PROWS], BF16, isOutput=False)
    fx = nc.declare_dram_parameter("fx", [128, ELW], BF16, isOutput=False)
    w1e = nc.declare_dram_parameter("w1e", [128, 128], BF16, isOutput=False)
    w2pack = nc.declare_dram_parameter("w2pack", [128, W2COLS], BF16, isOutput=False)
    out_d = nc.declare_dram_parameter("out", [OUT, E_C], F32, isOutput=True)

    with tile.TileContext(nc) as tc:
        _emit(nc, tc, fsp, fx, w1e, w2pack, out_d, alpha_a)
    nc.compile()
    return nc


def _emit(nc, tc, fsp, fx, w1e, w2pack, out_d, alpha):
    from contextlib import ExitStack

    Prelu = mybir.ActivationFunctionType.Prelu

    ctx = ExitStack()
    with ctx:
        const = ctx.enter_context(tc.tile_pool(name="const", bufs=1))
        work = ctx.enter_context(tc.tile_pool(name="work", bufs=1))
        psA = ctx.enter_context(tc.tile_pool(name="psA", bufs=3, space="PSUM"))
        psB = ctx.enter_context(tc.tile_pool(name="psB", bufs=2, space="PSUM"))
        psC = ctx.enter_context(tc.tile_pool(name="psC", bufs=1, space="PSUM"))

        # ---- persistent SBUF ----
        fsp_sb = const.tile([KROWS, PROWS], BF16)
        w1_sb = const.tile([128, 128], BF16)
        w2_sb = const.tile([128, W2COLS], BF16)
        h_sb = work.tile([128, PROWS], BF16)
        m_sb = work.tile([128, M_FREE], BF16)
        s2a = work.tile([128, S2A_FREE], BF16)
        s2e = work.tile([128, S2E_FREE], BF16)
        osb = work.tile([OUT, E_C], F32)

        # ---- PE warmup: junk matmuls to flip the HAM clock gate to 8/8
        # before real work arrives (reads a memset junk tile) ----
        junk = work.tile([128, 64], BF16)
        nc.vector.memset(junk[:].bitcast(F32), 0.0)
        pw = psC.tile([OUT, E_C], F32, tag="psC")
        for wi in range(80):
            nc.tensor.matmul(pw[0:64, 0:64], junk[:], junk[:],
                             start=True, stop=True)

        # w2 halves on SWDGE, issued before the gpsimd memsets (SWDGE
        # after engine-ops on this queue miscompiles); stage C needs
        # them late so the bandwidth hit is tolerable.
        nc.gpsimd.dma_start(w2_sb[:, W2COLS // 2 :], w2pack.ap()[:, W2COLS // 2 :])
        nc.gpsimd.dma_start(w2_sb[:, 0 : W2COLS // 2], w2pack.ap()[:, 0 : W2COLS // 2])

        # ---- memsets for M off-diagonal zeros, 4 el-aligned pieces so
        # the el-DMAs can chase them (vector + gpsimd, f32 view) ----
        m_f32 = m_sb[:].bitcast(F32)
        P4 = M_FREE // 8            # f32 cols per piece (2 el blocks)
        nc.vector.memset(m_f32[:, 0 * P4 : 1 * P4], 0.0)   # els 0,1
        nc.gpsimd.memset(m_f32[:, 1 * P4 : 2 * P4], 0.0)   # els 2,3
        nc.vector.memset(m_f32[:, 2 * P4 : 3 * P4], 0.0)   # els 4,5
        nc.gpsimd.memset(m_f32[:, 3 * P4 : 4 * P4], 0.0)   # els 6,7

        # w1 + all fsp chunks on the scalar ring: the only early HWDGE
        # traffic besides the el-DMAs, so stage A unblocks fast.
        CHW = PROWS // 4
        nc.scalar.dma_start(w1_sb[:], w1e.ap())
        for c in range(4):
            nc.scalar.dma_start(
                fsp_sb[:, c * CHW : (c + 1) * CHW],
                fsp.ap()[:, c * CHW : (c + 1) * CHW])


        # fused M diag el-DMAs: one per el; dst partitions are the
        # permuted el rows (stride-8 partition dim), cols contiguous.
        def el_dma(eng, el):
            return eng.dma_start(
                _ap(m_sb, el * (M_FREE + ELW), [
                    (8 * M_FREE, PITCH),    # i -> partition 8i+el
                    (1, ELW),
                ]),
                fx.ap()[el * PITCH : (el + 1) * PITCH, :])

        for el in range(EPG):
            el_dma(nc.sync, el)

        # ---- pipeline ----
        copy_rr = [0]

        def evac(dst, src):
            if copy_rr[0] % 3 == 2:
                nc.scalar.copy(dst, src)
            else:
                nc.vector.tensor_copy(dst, src)
            copy_rr[0] += 1

        def a_stage(sub):
            pa = psA.tile([128, SUBG * 128], F32, tag="psA")
            for j in range(SUBG):
                g = sub * SUBG + j
                nc.tensor.matmul(
                    pa[:, j * 128 : (j + 1) * 128],
                    fsp_sb[:, g * 128 : (g + 1) * 128],
                    w1_sb[0:KROWS, :],
                    start=True, stop=True)
            nc.scalar.activation(
                h_sb[:, sub * 512 : (sub + 1) * 512], pa[:],
                Prelu, scale=1.0, alpha=alpha)

        def b_ally(g0):              # 4 groups per 2-bank PSUM tile
            pb = psB.tile([128, 1024], F32, tag="psB")
            for u in range(4):
                g = g0 + u
                lhsT = h_sb[:, g * 128 : g * 128 + 64]
                nc.tensor.matmul(
                    pb[64:128, u * 192 : (u + 1) * 192], lhsT,
                    _ap(m_sb, g * ALLY_F + PAIR_A,
                        [(M_FREE, 128), (ELW, EPG), (1, PAIR_A)]),
                    start=True, stop=True)
                nc.tensor.matmul(
                    pb[0:64, u * 192 : (u + 1) * 192], lhsT,
                    _ap(m_sb, g * ALLY_F,
                        [(M_FREE, 128), (ELW, EPG), (1, PAIR_A)]),
                    start=True, stop=True)
            evac(
                _ap(s2a, g0 * EPG,
                    [(S2A_FREE, 128), (E_C, PAIR_A), (1, 4 * EPG)]),
                _ap(pb, 0,
                    [(1024, 128), (1, PAIR_A), (192, 4), (PAIR_A, EPG)]))

        def b_enemy(g0):             # 8 groups per 2-bank PSUM tile
            pb = psB.tile([128, 1024], F32, tag="psB")
            for u in range(8):
                g = g0 + u
                lhsT = h_sb[:, g * 128 + 64 : g * 128 + 128]
                nc.tensor.matmul(
                    pb[64:128, u * 128 : (u + 1) * 128], lhsT,
                    _ap(m_sb, EOFF + g * ENEMY_F + PAIR_E,
                        [(M_FREE, 128), (ELW, EPG), (1, PAIR_E)]),
                    start=True, stop=True)
                nc.tensor.matmul(
                    pb[0:64, u * 128 : (u + 1) * 128], lhsT,
                    _ap(m_sb, EOFF + g * ENEMY_F,
                        [(M_FREE, 128), (ELW, EPG), (1, PAIR_E)]),
                    start=True, stop=True)
            evac(
                _ap(s2e, g0 * EPG,
                    [(S2E_FREE, 128), (E_C, PAIR_E), (1, 8 * EPG)]),
                _ap(pb, 0,
                    [(1024, 128), (1, PAIR_E), (128, 8), (PAIR_E, EPG)]))

        for sub in range(NG // SUBG):
            a_stage(sub)
        for g0 in range(0, NG, 4):
            b_ally(g0)
        for g0 in range(0, NG, 8):
            b_enemy(g0)

        # ---- stage C: out_T[o,e] accumulation over 40 fp slices; slices
        # in the early-loaded w2 half (cols 2560:) run first ----
        pc = psC.tile([OUT, E_C], F32, tag="psC")
        n_sl = PAIR_A + PAIR_E
        order = ([("a", fp) for fp in range(PAIR_A)]
                 + [("e", fp) for fp in range(PAIR_E)])
        for idx, (br, fp) in enumerate(order):
            if br == "a":
                lhsT = w2_sb[:, fp * OUT : (fp + 1) * OUT]
                rhs = s2a[:, fp * E_C : (fp + 1) * E_C]
            else:
                lhsT = w2_sb[:, (PAIR_A + fp) * OUT : (PAIR_A + fp + 1) * OUT]
                rhs = s2e[:, fp * E_C : (fp + 1) * E_C]
            nc.tensor.matmul(pc[:], lhsT, rhs,
                             start=(idx == 0), stop=(idx == n_sl - 1))

        nc.vector.tensor_copy(osb[:], pc[:])
        nc.sync.dma_start(out_d.ap(), osb[:])


@functools.lru_cache(maxsize=2)
def _cached_program(alpha_a, alpha_e):
    return build_program(alpha_a, alpha_e)


def host_prep(ally_features, enemy_features, Wa1, ba1, aa, Wa2, ba2,
              We1, be1, ae, We2, be2):
    """Per-core input maps (numpy, bf16) + host-side bias term."""
    bf = ml_dtypes.bfloat16

    def uniform_alpha(a):
        a = np.asarray(a, dtype=np.float32)
        assert np.allclose(a, a[0]), "per-channel alpha not supported"
        return float(a[0])

    ua, ue = uniform_alpha(aa), uniform_alpha(ae)

    w1 = np.zeros((128, 128), dtype=np.float32)
    w1[0:ALLY_F, 0:HYPER] = np.asarray(Wa1)
    w1[ALLY_F:80, HYPER:128] = np.asarray(We1)
    w1[80, 0:HYPER] = np.asarray(ba1)
    w1[80, HYPER:128] = np.asarray(be1)
    w1 = w1.astype(bf)

    w2 = np.zeros((128, W2COLS), dtype=np.float32)
    Wa2_, We2_ = np.asarray(Wa2), np.asarray(We2)
    for fp in range(PAIR_A):
        w2[0:HYPER, fp * OUT : (fp + 1) * OUT] = Wa2_[:, fp * OUT : (fp + 1) * OUT]
        w2[HYPER:128, fp * OUT : (fp + 1) * OUT] = \
            Wa2_[:, (fp + PAIR_A) * OUT : (fp + PAIR_A + 1) * OUT]
    for fp in range(PAIR_E):
        c = (PAIR_A + fp) * OUT
        w2[0:HYPER, c : c + OUT] = We2_[:, fp * OUT : (fp + 1) * OUT]
        w2[HYPER:128, c : c + OUT] = \
            We2_[:, (fp + PAIR_E) * OUT : (fp + PAIR_E + 1) * OUT]
    w2 = w2.astype(bf)

    fa_all = np.asarray(ally_features, dtype=np.float32)
    fe_all = np.asarray(enemy_features, dtype=np.float32)
    fa_bf = fa_all.astype(bf)
    fe_bf = fe_all.astype(bf)

    # host-side bias term: fsum @ b2 (exact fp32)
    fsum_a = fa_all.reshape(B_FULL, N_AGENTS, ALLY_F).sum(axis=1)
    fsum_e = fe_all.reshape(B_FULL, N_ENEMIES, ENEMY_F).sum(axis=1)
    bias_out = (fsum_a @ np.asarray(ba2).reshape(ALLY_F, OUT)
                + fsum_e @ np.asarray(be2).reshape(ENEMY_F, OUT)).astype(np.float32)

    RA = E_C * N_AGENTS
    RE = E_C * N_ENEMIES
    in_maps = []
    for c in range(N_CORES):
        fa_c = fa_bf[c * RA : (c + 1) * RA].reshape(E_C, N_AGENTS, ALLY_F)
        fe_c = fe_bf[c * RE : (c + 1) * RE].reshape(E_C, N_ENEMIES, ENEMY_F)
        fa_pad = np.zeros((E_C, PITCH, ALLY_F), dtype=bf)
        fa_pad[:, :N_AGENTS, :] = fa_c
        fe_pad = np.zeros((E_C, PITCH, ENEMY_F), dtype=bf)
        fe_pad[:, :N_ENEMIES, :] = fe_c

        # group-row layouts under the partition permutation:
        # within group g, partition p holds episode-row (el,i) = PERM_ROW[p]
        def group_rows(f_pad, featf):
            f5 = f_pad.reshape(NG, EPG, PITCH, featf)
            out = np.empty((NG, 128, featf), dtype=bf)
            for p in range(128):
                el, i = PERM_ROW[p]
                out[:, p, :] = f5[:, el, i, :]
            return out

        ga = group_rows(fa_pad, ALLY_F)          # [NG, 128, 48]
        ge = group_rows(fe_pad, ENEMY_F)         # [NG, 128, 32]

        # fsp: [81, PROWS], col g*128 + p
        fs = np.zeros((KROWS, PROWS), dtype=bf)
        fs[0:ALLY_F, :] = ga.transpose(2, 0, 1).reshape(ALLY_F, PROWS)
        fs[ALLY_F:80, :] = ge.transpose(2, 0, 1).reshape(ENEMY_F, PROWS)
        fs[80, :] = np.float32(1.0)

        # fx: compact el-major diag source; row el*16 + i
        def m_compact(f_pad, featf):
            f5 = f_pad.reshape(NG, EPG, PITCH, featf)
            out = np.empty((128, NG * featf), dtype=bf)
            for el in range(EPG):
                for i in range(PITCH):
                    out[el * PITCH + i] = f5[:, el, i, :].reshape(NG * featf)
            return out

        in_maps.append({
            "fsp": np.ascontiguousarray(fs),
            "fx": np.ascontiguousarray(np.concatenate(
                [m_compact(fa_pad, ALLY_F), m_compact(fe_pad, ENEMY_F)],
                axis=1)),
            "w1e": w1, "w2pack": w2,
        })
    aux = {"bias_out": bias_out, "ua": ua, "ue": ue}
    return in_maps, aux


def assemble_output(results, aux):
    outs = [np.asarray(r["out"], dtype=np.float32) for r in results]
    dev = np.concatenate([o.T for o in outs], axis=0)
    return dev + aux["bias_out"]


def kernel(**inputs) -> np.ndarray:
    in_maps, aux = host_prep(**inputs)
    nc = _cached_program(aux["ua"], aux["ue"])
    res = run_bass_kernel_spmd(nc, in_maps, core_ids=list(range(N_CORES)))
    return assemble_output(res.results, aux)


if __name__ == "__main__":
    build_program()
    print("built ok")


# revision 34
# speedup vs baseline: 1.0906x; 1.0220x over previous
"""Trainium2 Bass kernel for nn_APIHyperInputLayer (hypernet MLP, 8-core data parallel).

Math (per branch):
    h   = prelu(F @ W1 + b1, alpha)                       [R, 64]
    w   = (h @ W2 + b2).reshape(R, F, 128)
    hid = einsum('rf,rfo->ro', F, w)
    out = hid.reshape(E, n, 128).sum(1)                   [E, 128]

Restructured: S[k,e,f] = sum_i h[(e,i),k] F[(e,i),f]; out[e,o] =
sum_{k,f} S[k,e,f] W2[k,f*128+o] + (bias term, computed on host).

v10 schedule (row-major h; no transposes; el-major fused M):
  Episodes padded to PITCH=16 rows; 8 episodes = one 128-partition group;
  within-group row->partition permutation p = 8*i + el spreads each el's
  16 rows over stride-8 partitions (better SDMA engine coverage).
  A: per group g, matmul([128 rows, 128 k], lhsT=fsp[:, g*128:+128],
     rhs=w1ext[0:81]); fsp has a ones-row so the bias rides the matmul.
     PReLU (pure, alpha) evacuates PSUM->SBUF per 4 groups.
  M: fused [128, 8*2560] tile; memsets zero it in 4 el-aligned pieces;
     8 clean el-DMAs from the compact host array fx write the diagonal.
  B: per group, two 64-part matmuls (f-pair lo/hi in partition halves)
     per branch; 4 ally groups or 8 enemy groups share one 2-bank PSUM
     tile; evac casts PSUM->s2[kk, fp*256+e] (vector 2/3, scalar 1/3).
  C: 40 accumulating matmuls out_T[o,e] += W2pair_fp.T @ s2 slice; the
     slices living in the early-loaded w2 half run first.
Output per core: [128 o, 256 e] fp32; host transposes/concats and adds the
bias term fsum @ b2 (host numpy, exact fp32).
"""

import os
import sys
import functools

import numpy as np

for _p in ("/opt/trn_rl_repo", os.path.expanduser("~/.axon_site/_ro/trn_rl_repo")):
    if os.path.isdir(_p) and _p not in sys.path:
        sys.path.insert(0, _p)

import dataclasses

import ml_dtypes

import concourse.bass as bass
import concourse.bacc as bacc
import concourse.mybir as mybir
import concourse.tile as tile
from concourse.bass_utils import run_bass_kernel_spmd

BF16 = mybir.dt.bfloat16
F32 = mybir.dt.float32

# Problem constants (hardcoded per contest rules)
N_CORES = 8
N_AGENTS, N_ENEMIES = 10, 11
ALLY_F, ENEMY_F = 48, 32
HYPER = 64
OUT = 128
B_FULL = 2048
E_C = B_FULL // N_CORES            # episodes per core = 256

PITCH = 16                         # padded rows per episode
EPG = 8                            # episodes per group (8*16=128 partitions)
NG = E_C // EPG                    # 32 groups
PROWS = E_C * PITCH                # padded rows per core = 4096
PAIR_A = ALLY_F // 2               # 24
PAIR_E = ENEMY_F // 2              # 16
S2A_FREE = PAIR_A * E_C            # 6144
S2E_FREE = PAIR_E * E_C            # 4096
W2COLS = (PAIR_A + PAIR_E) * OUT   # 5120
KROWS = ALLY_F + ENEMY_F + 1       # 81 = stacked features + ones row
ELW = NG * ALLY_F + NG * ENEMY_F   # 2560 = fused el-block width (ally|enemy)
M_FREE = EPG * ELW                 # 20480 = fused M tile free dim
EOFF = NG * ALLY_F                 # 1536 = enemy col offset within el block
SUBG = 4                           # groups per psA tile / prelu


def _perm_p(el, i):
    """Within-group row -> partition: p = 8*i + el (stride-8 spread)."""
    return 8 * i + el


PERM_ROW = [None] * 128
for _el in range(EPG):
    for _i in range(PITCH):
        PERM_ROW[_perm_p(_el, _i)] = (_el, _i)


def _ap(t, offset, dims):
    """Custom flat AP: dims = [(step, num), ...]; t is an AP or tensor handle."""
    a = t if isinstance(t, bass.AP) else t.ap()
    return dataclasses.replace(a, offset=offset, ap=[[s, n] for (s, n) in dims])


def build_program(alpha_a=0.25, alpha_e=0.25):
    assert alpha_a == alpha_e, "branches must share alpha"
    nc = bacc.Bacc("TRN2", target_bir_lowering=False, debug=False)

    fsp = nc.declare_dram_parameter("fsp", [KROWS, PROWS], BF16, isOutput=False)
    fx = nc.declare_dram_parameter("fx", [128, ELW], BF16, isOutput=False)
    w1e = nc.declare_dram_parameter("w1e", [128, 128], BF16, isOutput=False)
    w2pack = nc.declare_dram_parameter("w2pack", [128, W2COLS], BF16, isOutput=False)
    out_d = nc.declare_dram_parameter("out", [OUT, E_C], F32, isOutput=True)

    with tile.TileContext(nc) as tc:
        _emit(nc, tc, fsp, fx, w1e, w2pack, out_d, alpha_a)
    nc.compile()
    return nc


def _emit(nc, tc, fsp, fx, w1e, w2pack, out_d, alpha):
    from contextlib import ExitStack

    Prelu = mybir.ActivationFunctionType.Prelu

    ctx = ExitStack()
    with ctx:
        const = ctx.enter_context(tc.tile_pool(name="const", bufs=1))
        work = ctx.enter_context(tc.tile_pool(name="work", bufs=1))
        psA = ctx.enter_context(tc.tile_pool(name="psA", bufs=3, space="PSUM"))
        psB = ctx.enter_context(tc.tile_pool(name="psB", bufs=2, space="PSUM"))
        psC = ctx.enter_context(tc.tile_pool(name="psC", bufs=1, space="PSUM"))

        # ---- persistent SBUF ----
        fsp_sb = const.tile([KROWS, PROWS], BF16)
        w1_sb = const.tile([128, 128], BF16)
        w2_sb = const.tile([128, W2COLS], BF16)
        h_sb = work.tile([128, PROWS], BF16)
        m_sb = work.tile([128, M_FREE], BF16)
        s2a = work.tile([128, S2A_FREE], BF16)
        s2e = work.tile([128, S2E_FREE], BF16)
        osb = work.tile([OUT, E_C], F32)

        # ---- PE warmup: junk matmuls to flip the HAM clock gate to 8/8
        # before real work arrives (reads a memset junk tile) ----
        junk = work.tile([128, 64], BF16)
        nc.vector.memset(junk[:].bitcast(F32), 0.0)
        pw = psC.tile([OUT, E_C], F32, tag="psC")
        for wi in range(80):
            nc.tensor.matmul(pw[0:64, 0:64], junk[:], junk[:],
                             start=True, stop=True)

        # w2 second half on SWDGE, issued before the gpsimd memsets
        # (SWDGE after engine-ops on this queue miscompiles).
        nc.gpsimd.dma_start(w2_sb[:, W2COLS // 2 :], w2pack.ap()[:, W2COLS // 2 :])

        # ---- memsets for M off-diagonal zeros, 4 el-aligned pieces so
        # the el-DMAs can chase them (vector + gpsimd, f32 view) ----
        m_f32 = m_sb[:].bitcast(F32)
        P4 = M_FREE // 8            # f32 cols per piece (2 el blocks)
        nc.vector.memset(m_f32[:, 0 * P4 : 1 * P4], 0.0)   # els 0,1
        nc.gpsimd.memset(m_f32[:, 1 * P4 : 2 * P4], 0.0)   # els 2,3
        nc.vector.memset(m_f32[:, 2 * P4 : 3 * P4], 0.0)   # els 4,5
        nc.gpsimd.memset(m_f32[:, 3 * P4 : 4 * P4], 0.0)   # els 6,7

        # w1 + all fsp chunks on the scalar ring: the only early HWDGE
        # traffic besides the el-DMAs, so stage A unblocks fast.
        CHW = PROWS // 4
        nc.sync.dma_start(fsp_sb[:, 0:CHW], fsp.ap()[:, 0:CHW])
        nc.sync.dma_start(fsp_sb[:, CHW : 2 * CHW], fsp.ap()[:, CHW : 2 * CHW])
        nc.scalar.dma_start(w1_sb[:], w1e.ap())
        nc.scalar.dma_start(
            fsp_sb[:, 2 * CHW : 3 * CHW], fsp.ap()[:, 2 * CHW : 3 * CHW])
        nc.scalar.dma_start(
            fsp_sb[:, 3 * CHW : 4 * CHW], fsp.ap()[:, 3 * CHW : 4 * CHW])


        # fused M diag el-DMAs: one per el; dst partitions are the
        # permuted el rows (stride-8 partition dim), cols contiguous.
        def el_dma(eng, el):
            return eng.dma_start(
                _ap(m_sb, el * (M_FREE + ELW), [
                    (8 * M_FREE, PITCH),    # i -> partition 8i+el
                    (1, ELW),
                ]),
                fx.ap()[el * PITCH : (el + 1) * PITCH, :])

        for el in range(EPG):
            el_dma(nc.sync, el)

        # w2 first half trails the el-DMAs on the sync ring
        nc.sync.dma_start(w2_sb[:, 0 : W2COLS // 2], w2pack.ap()[:, 0 : W2COLS // 2])

        # ---- pipeline ----
        copy_rr = [0]

        def evac(dst, src):
            if copy_rr[0] % 3 == 2:
                nc.scalar.copy(dst, src)
            else:
                nc.vector.tensor_copy(dst, src)
            copy_rr[0] += 1

        def a_stage(sub):
            pa = psA.tile([128, SUBG * 128], F32, tag="psA")
            for j in range(SUBG):
                g = sub * SUBG + j
                nc.tensor.matmul(
                    pa[:, j * 128 : (j + 1) * 128],
                    fsp_sb[:, g * 128 : (g + 1) * 128],
                    w1_sb[0:KROWS, :],
                    start=True, stop=True)
            nc.scalar.activation(
                h_sb[:, sub * 512 : (sub + 1) * 512], pa[:],
                Prelu, scale=1.0, alpha=alpha)

        def b_ally(g0):              # 4 groups per 2-bank PSUM tile
            pb = psB.tile([128, 1024], F32, tag="psB")
            for u in range(4):
                g = g0 + u
                lhsT = h_sb[:, g * 128 : g * 128 + 64]
                nc.tensor.matmul(
                    pb[64:128, u * 192 : (u + 1) * 192], lhsT,
                    _ap(m_sb, g * ALLY_F + PAIR_A,
                        [(M_FREE, 128), (ELW, EPG), (1, PAIR_A)]),
                    start=True, stop=True)
                nc.tensor.matmul(
                    pb[0:64, u * 192 : (u + 1) * 192], lhsT,
                    _ap(m_sb, g * ALLY_F,
                        [(M_FREE, 128), (ELW, EPG), (1, PAIR_A)]),
                    start=True, stop=True)
            evac(
                _ap(s2a, g0 * EPG,
                    [(S2A_FREE, 128), (E_C, PAIR_A), (1, 4 * EPG)]),
                _ap(pb, 0,
                    [(1024, 128), (1, PAIR_A), (192, 4), (PAIR_A, EPG)]))

        def b_enemy(g0):             # 8 groups per 2-bank PSUM tile
            pb = psB.tile([128, 1024], F32, tag="psB")
            for u in range(8):
                g = g0 + u
                lhsT = h_sb[:, g * 128 + 64 : g * 128 + 128]
                nc.tensor.matmul(
                    pb[64:128, u * 128 : (u + 1) * 128], lhsT,
                    _ap(m_sb, EOFF + g * ENEMY_F + PAIR_E,
                        [(M_FREE, 128), (ELW, EPG), (1, PAIR_E)]),
                    start=True, stop=True)
                nc.tensor.matmul(
                    pb[0:64, u * 128 : (u + 1) * 128], lhsT,
                    _ap(m_sb, EOFF + g * ENEMY_F,
                        [(M_FREE, 128), (ELW, EPG), (1, PAIR_E)]),
                    start=True, stop=True)
            evac(
                _ap(s2e, g0 * EPG,
                    [(S2E_FREE, 128), (E_C, PAIR_E), (1, 8 * EPG)]),
                _ap(pb, 0,
                    [(1024, 128), (1, PAIR_E), (128, 8), (PAIR_E, EPG)]))

        for sub in range(NG // SUBG):
            a_stage(sub)
        for g0 in range(0, NG, 4):
            b_ally(g0)
        for g0 in range(0, NG, 8):
            b_enemy(g0)

        # ---- stage C: out_T[o,e] accumulation over 40 fp slices; slices
        # in the early-loaded w2 half (cols 2560:) run first ----
        pc = psC.tile([OUT, E_C], F32, tag="psC")
        n_sl = PAIR_A + PAIR_E
        order = ([("a", fp) for fp in range(PAIR_A)]
                 + [("e", fp) for fp in range(PAIR_E)])
        for idx, (br, fp) in enumerate(order):
            if br == "a":
                lhsT = w2_sb[:, fp * OUT : (fp + 1) * OUT]
                rhs = s2a[:, fp * E_C : (fp + 1) * E_C]
            else:
                lhsT = w2_sb[:, (PAIR_A + fp) * OUT : (PAIR_A + fp + 1) * OUT]
                rhs = s2e[:, fp * E_C : (fp + 1) * E_C]
            nc.tensor.matmul(pc[:], lhsT, rhs,
                             start=(idx == 0), stop=(idx == n_sl - 1))

        nc.vector.tensor_copy(osb[:], pc[:])
        nc.sync.dma_start(out_d.ap(), osb[:])


@functools.lru_cache(maxsize=2)
def _cached_program(alpha_a, alpha_e):
    return build_program(alpha_a, alpha_e)


def host_prep(ally_features, enemy_features, Wa1, ba1, aa, Wa2, ba2,
              We1, be1, ae, We2, be2):
    """Per-core input maps (numpy, bf16) + host-side bias term."""
    bf = ml_dtypes.bfloat16

    def uniform_alpha(a):
        a = np.asarray(a, dtype=np.float32)
        assert np.allclose(a, a[0]), "per-channel alpha not supported"
        return float(a[0])

    ua, ue = uniform_alpha(aa), uniform_alpha(ae)

    w1 = np.zeros((128, 128), dtype=np.float32)
    w1[0:ALLY_F, 0:HYPER] = np.asarray(Wa1)
    w1[ALLY_F:80, HYPER:128] = np.asarray(We1)
    w1[80, 0:HYPER] = np.asarray(ba1)
    w1[80, HYPER:128] = np.asarray(be1)
    w1 = w1.astype(bf)

    w2 = np.zeros((128, W2COLS), dtype=np.float32)
    Wa2_, We2_ = np.asarray(Wa2), np.asarray(We2)
    for fp in range(PAIR_A):
        w2[0:HYPER, fp * OUT : (fp + 1) * OUT] = Wa2_[:, fp * OUT : (fp + 1) * OUT]
        w2[HYPER:128, fp * OUT : (fp + 1) * OUT] = \
            Wa2_[:, (fp + PAIR_A) * OUT : (fp + PAIR_A + 1) * OUT]
    for fp in range(PAIR_E):
        c = (PAIR_A + fp) * OUT
        w2[0:HYPER, c : c + OUT] = We2_[:, fp * OUT : (fp + 1) * OUT]
        w2[HYPER:128, c : c + OUT] = \
            We2_[:, (fp + PAIR_E) * OUT : (fp + PAIR_E + 1) * OUT]
    w2 = w2.astype(bf)

    fa_all = np.asarray(ally_features, dtype=np.float32)
    fe_all = np.asarray(enemy_features, dtype=np.float32)
    fa_bf = fa_all.astype(bf)
    fe_bf = fe_all.astype(bf)

    # host-side bias term: fsum @ b2 (exact fp32)
    fsum_a = fa_all.reshape(B_FULL, N_AGENTS, ALLY_F).sum(axis=1)
    fsum_e = fe_all.reshape(B_FULL, N_ENEMIES, ENEMY_F).sum(axis=1)
    bias_out = (fsum_a @ np.asarray(ba2).reshape(ALLY_F, OUT)
                + fsum_e @ np.asarray(be2).reshape(ENEMY_F, OUT)).astype(np.float32)

    RA = E_C * N_AGENTS
    RE = E_C * N_ENEMIES
    in_maps = []
    for c in range(N_CORES):
        fa_c = fa_bf[c * RA : (c + 1) * RA].reshape(E_C, N_AGENTS, ALLY_F)
        fe_c = fe_bf[c * RE : (c + 1) * RE].reshape(E_C, N_ENEMIES, ENEMY_F)
        fa_pad = np.zeros((E_C, PITCH, ALLY_F), dtype=bf)
        fa_pad[:, :N_AGENTS, :] = fa_c
        fe_pad = np.zeros((E_C, PITCH, ENEMY_F), dtype=bf)
        fe_pad[:, :N_ENEMIES, :] = fe_c

        # group-row layouts under the partition permutation:
        # within group g, partition p holds episode-row (el,i) = PERM_ROW[p]
        def group_rows(f_pad, featf):
            f5 = f_pad.reshape(NG, EPG, PITCH, featf)
            out = np.empty((NG, 128, featf), dtype=bf)
            for p in range(128):
                el, i = PERM_ROW[p]
                out[:, p, :] = f5[:, el, i, :]
            return out

        ga = group_rows(fa_pad, ALLY_F)          # [NG, 128, 48]
        ge = group_rows(fe_pad, ENEMY_F)         # [NG, 128, 32]

        # fsp: [81, PROWS], col g*128 + p
        fs = np.zeros((KROWS, PROWS), dtype=bf)
        fs[0:ALLY_F, :] = ga.transpose(2, 0, 1).reshape(ALLY_F, PROWS)
        fs[ALLY_F:80, :] = ge.transpose(2, 0, 1).reshape(ENEMY_F, PROWS)
        fs[80, :] = np.float32(1.0)

        # fx: compact el-major diag source; row el*16 + i
        def m_compact(f_pad, featf):
            f5 = f_pad.reshape(NG, EPG, PITCH, featf)
            out = np.empty((128, NG * featf), dtype=bf)
            for el in range(EPG):
                for i in range(PITCH):
                    out[el * PITCH + i] = f5[:, el, i, :].reshape(NG * featf)
            return out

        in_maps.append({
            "fsp": np.ascontiguousarray(fs),
            "fx": np.ascontiguousarray(np.concatenate(
                [m_compact(fa_pad, ALLY_F), m_compact(fe_pad, ENEMY_F)],
                axis=1)),
            "w1e": w1, "w2pack": w2,
        })
    aux = {"bias_out": bias_out, "ua": ua, "ue": ue}
    return in_maps, aux


def assemble_output(results, aux):
    outs = [np.asarray(r["out"], dtype=np.float32) for r in results]
    dev = np.concatenate([o.T for o in outs], axis=0)
    return dev + aux["bias_out"]


def kernel(**inputs) -> np.ndarray:
    in_maps, aux = host_prep(**inputs)
    nc = _cached_program(aux["ua"], aux["ue"])
    res = run_bass_kernel_spmd(nc, in_maps, core_ids=list(range(N_CORES)))
    return assemble_output(res.results, aux)


if __name__ == "__main__":
    build_program()
    print("built ok")
